# revision 3
# baseline (speedup 1.0000x reference)
"""TRN2 Bass kernel for nn_Attention_56281251447235 (v3).

Multi-head attention: x:[4,2048,1024], w_qkv:[1024,3072] (q|k|v),
16 heads x 64 dim_head, w_out:[1024,1024], b_out:[1024].

Sharding over 8 NeuronCores: core j handles batch b=j//2 and head-group
hg=j%2 (8 of 16 heads).  Each core computes its 8 heads' attention and the
head-group's output projection [2048,1024] in f16; the host sums the two
head-group arrays per batch (plus a small tail-residual array) and adds
the bias.

v3 changes over v2 (cost-model driven):
  * lookahead ST: the QK matmul for position g+1 is emitted before PV(g),
    so the exp stream on ACT never waits on a just-in-time matmul.
  * dummy activation at t=0 preloads the exp table set off the critical
    path (-1.3us).
  * batched, host-packed input DMAs (pair-packed wq/wk/wv with 2KB
    descriptor runs; x in eighth/quarter chunks) - v2's 117 DMAs at
    625ns HWDGE each starved the early pipeline.
  * v-projection merged per (pair, tci): 8 matmuls of 128-wide instead of
    16 of 64-wide, one strided DVE copy into v_aug.
  * single f16 output: the fc3 (pair-3) projection adds in-place into the
    staged A-part (fc0-2) on-device for tokens 0-1023; for tokens
    1024-2047 the fc3 part ships as a separate small f16 residual so the
    tail is not serialized behind DVE adds.  Stores are batched (4-tci
    groups, 1-2MB per DMA).
  * tail: per-ic pipeline of the last transpose -> fc3 projection, with
    psum->sbuf copies spread over ACT/DVE/Pool (ACT is idle after the
    last exp).
"""

import numpy as np
from ml_dtypes import bfloat16

import concourse.mybir as mybir
import concourse.tile as tile
from concourse import bacc
from concourse.bass_utils import run_bass_kernel_spmd

F32 = mybir.dt.float32
F16 = mybir.dt.float16
BF16 = mybir.dt.bfloat16
EXP = mybir.ActivationFunctionType.Exp

P = 128
B, N, DIM = 4, 2048, 1024
H_LOC = 8  # heads per core
D = 64  # dim per head
FEAT = H_LOC * D  # 512 inner dims per core
KC = DIM // P  # 8 contraction chunks over model dim
NT = N // P  # 16 token chunks
FC = FEAT // P  # 4 feature chunks
IB = 1024  # attention i-block width
NIB = N // IB  # 2
NPAIR = H_LOC // 2  # 4
SCALE = 1.0 / 8.0  # dim_head ** -0.5

_CACHE = {}

import os as _os

DRIP_B1 = int(_os.environ.get("DRIP_B1", "2700"))
DRIP_B2 = int(_os.environ.get("DRIP_B2", "1800"))
DRIP_JC = int(_os.environ.get("DRIP_JC", "1024"))


def _emit(nc, tc, xT_d, wq_d, wk_d, wv_d, wo_d, id_d, out_d, outr_d):
    from collections import deque
    from contextlib import ExitStack

    with ExitStack() as ctx:
        big = ctx.enter_context(tc.tile_pool(name="big", bufs=1))
        mm512 = ctx.enter_context(tc.tile_pool(name="mm512", bufs=2, space="PSUM"))
        ps_st = ctx.enter_context(tc.tile_pool(name="ps_st", bufs=2, space="PSUM"))
        ps_ot = ctx.enter_context(tc.tile_pool(name="ps_ot", bufs=1, space="PSUM"))
        pb2 = ctx.enter_context(tc.tile_pool(name="pb2", bufs=4))  # qT/kT, all pairs
        pb1 = ctx.enter_context(tc.tile_pool(name="pb1", bufs=1))  # weights
        pbe = ctx.enter_context(tc.tile_pool(name="pbe", bufs=4))  # ex ring

        # ---- persistent tiles ----
        xT = big.tile([P, KC, N], BF16)  # 32KB/p
        v_aug = big.tile([P, NT, H_LOC, D + 1], BF16)  # 16.6KB/p
        OT = big.tile([P, FC, N], BF16)  # 16KB/p
        ident = big.tile([P, P], BF16)
        rec_sb = big.tile([P, 2, 4, 1], F32)
        o_all = big.tile([P, NIB, FC, 8, 2, D], BF16)  # 16KB/p
        stage = big.tile([P, NT, DIM], F16)  # 32KB/p - final output staging
        stage2 = big.tile([P, NT // 2, DIM], F16)  # 16KB/p - fc3 tail residual
        dummy = big.tile([P, 2], F32)
        wv = pb1.tile([P, KC, FEAT], BF16, tag="wv")
        wo = pb1.tile([P, FC, DIM], BF16, tag="wo")

        # dummy activation: pulls the exp table load to t=0 (off the
        # critical path; the table DMA is ~1.3us)
        nc.vector.memset(dummy[:], 0.0)
        nc.scalar.activation(dummy[:, 1:2], dummy[:, 0:1], EXP, scale=1.0)

        # ones column of v_aug (65th col of every head) via f32 scratch
        with tc.tile_pool(name="init", bufs=1) as init:
            onec = init.tile([P, 1, 1], F32)
            nc.vector.memset(onec[:], 1.0)
            nc.vector.tensor_copy(
                v_aug[:, :, :, D], onec[:].to_broadcast([P, NT, H_LOC])
            )

        # ---- input DMAs (order = head-critical first) ----
        wqs, wks = [], []
        for pair in range(NPAIR):
            wqs.append(pb1.tile([P, KC, P], BF16, tag=f"wq{pair}", name=f"wq{pair}"))
            wks.append(pb1.tile([P, KC, P], BF16, tag=f"wk{pair}", name=f"wk{pair}"))
        xT_r = xT_d.ap().rearrange("(kc p) t -> p kc t", p=P)
        out_r = out_d.ap().rearrange("(tc p) o -> p tc o", p=P)
        outr_r = outr_d.ap().rearrange("(tc p) o -> p tc o", p=P)

        # head: small first pieces so the first k-proj matmul starts ~4us
        nc.sync.dma_start(wks[0][:, 0:2], wk_d.ap()[0, :, 0:2])
        nc.sync.dma_start(xT[:, 0:2, 0:512], xT_r[:, 0:2, 0:512])
        nc.sync.dma_start(wks[0][:, 2:8], wk_d.ap()[0, :, 2:8])
        nc.sync.dma_start(xT[:, 2:4, 0:512], xT_r[:, 2:4, 0:512])
        nc.sync.dma_start(wqs[0][:], wq_d.ap()[0])
        nc.sync.dma_start(xT[:, 4:8, 0:512], xT_r[:, 4:8, 0:512])
        nc.sync.dma_start(xT[:, 0:4, 512:1024], xT_r[:, 0:4, 512:1024])
        nc.sync.dma_start(xT[:, 4:8, 512:1024], xT_r[:, 4:8, 512:1024])
        nc.sync.dma_start(wv[:, :, 0:P], wv_d.ap()[0])  # pair-0 v weights
        nc.sync.dma_start(ident[:], id_d.ap())
        nc.sync.dma_start(xT[:, :, 1024:1536], xT_r[:, :, 1024:1536])
        nc.sync.dma_start(wv[:, :, P : 2 * P], wv_d.ap()[1])
        nc.sync.dma_start(wks[1][:], wk_d.ap()[1])
        nc.sync.dma_start(wqs[1][:], wq_d.ap()[1])
        nc.sync.dma_start(xT[:, :, 1536:2048], xT_r[:, :, 1536:2048])
        nc.sync.dma_start(wv[:, :, 2 * P : 3 * P], wv_d.ap()[2])
        nc.sync.dma_start(wks[2][:], wk_d.ap()[2])
        nc.sync.dma_start(wqs[2][:], wq_d.ap()[2])
        nc.sync.dma_start(wv[:, :, 3 * P : 4 * P], wv_d.ap()[3])
        nc.sync.dma_start(wks[3][:], wk_d.ap()[3])
        nc.sync.dma_start(wqs[3][:], wq_d.ap()[3])
        nc.sync.dma_start(wo[:], wo_d.ap())

        # ---- work units (generators; ~one PE matmul per yield) ----
        def proj_unit(dst, w, ib4):
            ps = mm512.tile([P, 512], F32, tag="mm512", name="ps")
            for kc in range(KC):
                nc.tensor.matmul(
                    ps[:],
                    w[:, kc],
                    xT[:, kc, ib4 * 512 : (ib4 + 1) * 512],
                    start=(kc == 0),
                    stop=(kc == KC - 1),
                )
                yield
            nc.vector.tensor_copy(dst[:, ib4 * 512 : (ib4 + 1) * 512], ps[:])

        def v_unit(pair, tci):
            # both heads of `pair` at once: moving 128 wide
            ps = mm512.tile([P, 2 * D], F32, tag="mm512", name="ps")
            for kc in range(KC):
                nc.tensor.matmul(
                    ps[:],
                    xT[:, kc, tci * P : (tci + 1) * P],
                    wv[:, kc, pair * P : (pair + 1) * P],
                    start=(kc == 0),
                    stop=(kc == KC - 1),
                )
                yield
            nc.vector.tensor_copy(
                v_aug[:, tci, 2 * pair : 2 * pair + 2, 0:D],
                ps[:].rearrange("p (h d) -> p h d", h=2),
            )

        def ca_unit(tci, nb):
            # A-part output projection (fc 0..2) -> staging (f16)
            ps = mm512.tile([P, 512], F32, tag="mm512", name="ps")
            for i, fc in enumerate((0, 1, 2)):
                nc.tensor.matmul(
                    ps[:],
                    OT[:, fc, tci * P : (tci + 1) * P],
                    wo[:, fc, nb * 512 : (nb + 1) * 512],
                    start=(i == 0),
                    stop=(i == 2),
                )
                yield
            nc.vector.tensor_copy(stage[:, tci, nb * 512 : (nb + 1) * 512], ps[:])

        def cb_unit(tci, nb):
            # fc3 projection, added in place into the staged A-part
            ps = mm512.tile([P, 512], F32, tag="mm512", name="ps")
            nc.tensor.matmul(
                ps[:],
                OT[:, 3, tci * P : (tci + 1) * P],
                wo[:, 3, nb * 512 : (nb + 1) * 512],
                start=True,
                stop=True,
            )
            yield
            sl = stage[:, tci, nb * 512 : (nb + 1) * 512]
            nc.vector.tensor_add(sl, ps[:], sl)

        def dma_unit(dst_r, src, t0, t1):
            yield
            nc.sync.dma_start(dst_r[:, t0:t1, :], src[:, t0:t1, :])

        def tp_unit(pair, ib):
            op = o_all[:, ib, pair]
            tp = mm512.tile([P, 8, P], BF16, tag="mm512", name="tp")
            for ic in range(8):
                nc.tensor.matmul(
                    tp[:, ic],
                    op[:, ic].rearrange("p a b -> p (a b)"),
                    ident[:],
                    is_transpose=True,
                    start=(ic == 0),
                    stop=(ic == 7),
                )
                yield
            nc.vector.tensor_copy(
                OT[:, pair, ib * IB : (ib + 1) * IB],
                tp[:].rearrange("p a b -> p (a b)"),
            )

        # fillers: FIFO of (key, generator, cycles-per-yield)
        fillers = deque()
        done_keys = set()

        def drip(budget=900):
            while budget > 0 and fillers:
                try:
                    next(fillers[0][1])
                    budget -= max(fillers[0][2], 1)
                except StopIteration:
                    done_keys.add(fillers[0][0])
                    fillers.popleft()

        def ensure(*keys):
            need = set(keys) - done_keys
            while need:
                key, gen, _ = fillers.popleft()
                for _ in gen:
                    pass
                done_keys.add(key)
                need.discard(key)

        def drain(gen):
            for _ in gen:
                pass

        qTs, kTs = [], []
        for pair in range(NPAIR):
            qTs.append(pb2.tile([P, N], BF16, tag="qT", name="qT"))
            kTs.append(pb2.tile([P, N], BF16, tag="kT", name="kT"))

        # ---- phase A: minimal eager prefix ----
        drain(proj_unit(kTs[0], wks[0], 0))
        drain(proj_unit(qTs[0], wqs[0], 0))
        drain(proj_unit(qTs[0], wqs[0], 1))
        done_keys.update({"k0.0", "q0.0", "q0.1"})

        # ---- drip queue, in consumption order ----
        def put_proj(key, dst, w, ib4):
            fillers.append((key, proj_unit(dst, w, ib4), 512))

        def put_pair(pair, with_q01):
            if with_q01:
                put_proj(f"k{pair}.0", kTs[pair], wks[pair], 0)
                put_proj(f"q{pair}.0", qTs[pair], wqs[pair], 0)
                put_proj(f"q{pair}.1", qTs[pair], wqs[pair], 1)
            for tci in range(4):
                fillers.append((f"v{pair}.{tci}", v_unit(pair, tci), 1024))
            put_proj(f"k{pair}.1", kTs[pair], wks[pair], 1)
            for tci in range(4, 8):
                fillers.append((f"v{pair}.{tci}", v_unit(pair, tci), 1024))
            put_proj(f"k{pair}.2", kTs[pair], wks[pair], 2)
            for tci in range(8, 12):
                fillers.append((f"v{pair}.{tci}", v_unit(pair, tci), 1024))
            put_proj(f"k{pair}.3", kTs[pair], wks[pair], 3)
            for tci in range(12, 16):
                fillers.append((f"v{pair}.{tci}", v_unit(pair, tci), 1024))
            put_proj(f"q{pair}.2", qTs[pair], wqs[pair], 2)
            put_proj(f"q{pair}.3", qTs[pair], wqs[pair], 3)

        for pair in range(NPAIR):
            put_pair(pair, with_q01=(pair > 0))

        # ---- main sweep: lookahead-ST + ACT-paced stream ----
        blocks = [(p, i, h) for p in range(NPAIR) for i in range(NIB) for h in range(2)]
        positions = [(bi, jc) for bi in range(len(blocks)) for jc in range(NT)]

        st_q = deque()

        def emit_st_for(bi, jc):
            pair, ib, h2 = blocks[bi]
            ensure(f"k{pair}.{jc // 4}", f"q{pair}.{2 * ib}", f"q{pair}.{2 * ib + 1}")
            st = ps_st.tile([P, IB], F32, tag="st", name="st")
            kh = kTs[pair][h2 * D : (h2 + 1) * D]
            qh = qTs[pair][h2 * D : (h2 + 1) * D]
            for hf in range(2):
                nc.tensor.matmul(
                    st[:, hf * 512 : (hf + 1) * 512],
                    kh[:, jc * P : (jc + 1) * P],
                    qh[:, ib * IB + hf * 512 : ib * IB + (hf + 1) * 512],
                    start=True,
                    stop=True,
                )
            st_q.append(st)

        emit_st_for(0, 0)

        pending_norm = None
        pending_tp = None
        ot_ps = None
        for idx, (bi, jc) in enumerate(positions):
            pair, ib, h2 = blocks[bi]
            h = 2 * pair + h2
            if jc == 0:
                if pending_norm is not None:
                    pending_norm()
                    pending_norm = None
                if pending_tp is not None:
                    fillers.append(pending_tp)
                    pending_tp = None
                if bi == 10:
                    for tci in range(8):
                        for nb in range(2):
                            fillers.append(
                                (f"ca{tci}.{nb}", ca_unit(tci, nb), 512)
                            )
                if bi == 12:
                    for tci in range(8, 12):
                        for nb in range(2):
                            fillers.append(
                                (f"ca{tci}.{nb}", ca_unit(tci, nb), 512)
                            )
                    fillers.append(("dmaA0", dma_unit(out_r, stage, 8, 12), 1))
                    for tci in range(12, 16):
                        for nb in range(2):
                            fillers.append(
                                (f"ca{tci}.{nb}", ca_unit(tci, nb), 512)
                            )
                    fillers.append(("dmaA1", dma_unit(out_r, stage, 12, 16), 1))
                if bi == 14:
                    for tci in range(4):
                        for nb in range(2):
                            fillers.append(
                                (f"cb{tci}.{nb}", cb_unit(tci, nb), 512)
                            )
                    fillers.append(("dmaB0", dma_unit(out_r, stage, 0, 4), 1))
                    for tci in range(4, 8):
                        for nb in range(2):
                            fillers.append(
                                (f"cb{tci}.{nb}", cb_unit(tci, nb), 512)
                            )
                    fillers.append(("dmaB1", dma_unit(out_r, stage, 4, 8), 1))
                ot_ps = ps_ot.tile([P, 2, 4, P], F32, tag="ot", name="ot_ps")

            # ACT: exp of the pre-emitted ST
            ex = pbe.tile([P, IB], BF16, tag="ex", name="ex", bufs=6)
            nc.scalar.activation(ex[:], st_q.popleft()[:], EXP, scale=SCALE)

            # PE: lookahead ST for the next position
            if idx + 1 < len(positions):
                emit_st_for(*positions[idx + 1])

            drip(DRIP_B1 if jc == 0 else (DRIP_B2 if jc == 1 else DRIP_JC))

            # PE: PV for this position
            ensure(f"v{pair}.{jc}")
            for ic in range(8):
                nc.tensor.matmul(
                    ot_ps[:, ic // 4, ic % 4, 0 : D + 1],
                    ex[:, ic * P : (ic + 1) * P],
                    v_aug[:, jc, h],
                    start=(jc == 0 and ic % 4 == 0),
                    stop=(jc == NT - 1 and ic % 4 == 3),
                )

            if jc == NT - 1:

                def _norm(ot_ps=ot_ps, ib=ib, pair=pair, h2=h2):
                    nc.vector.reciprocal(rec_sb[:], ot_ps[:, :, :, D : D + 1])
                    for b2 in range(2):
                        nc.vector.tensor_mul(
                            o_all[:, ib, pair, b2 * 4 : (b2 + 1) * 4, h2, :],
                            ot_ps[:, b2, :, 0:D],
                            rec_sb[:, b2].to_broadcast([P, 4, D]),
                        )

                pending_norm = _norm
                if h2 == 1:
                    pending_tp = (f"tp{pair}.{ib}", tp_unit(pair, ib), 128)

        # ---- tail: last norm, per-ic tp3.1 -> fc3 residual -> stage2 ----
        pending_norm()
        op = o_all[:, 1, 3]
        for ic in range(8):
            tp = mm512.tile([P, P], BF16, tag="mm512", name="tp")
            nc.tensor.matmul(
                tp[:],
                op[:, ic].rearrange("p a b -> p (a b)"),
                ident[:],
                is_transpose=True,
                start=True,
                stop=True,
            )
            otc = OT[:, 3, IB + ic * P : IB + (ic + 1) * P]
            nc.vector.tensor_copy(otc, tp[:])
            ps = ps_st.tile([P, IB], F32, tag="st", name="st")
            for nb in range(2):
                nc.tensor.matmul(
                    ps[:, nb * 512 : (nb + 1) * 512],
                    otc,
                    wo[:, 3, nb * 512 : (nb + 1) * 512],
                    start=True,
                    stop=True,
                )
            # copies alternate between ACT (idle now) and DVE
            if ic % 2 == 0:
                nc.scalar.copy(stage2[:, ic, 0:512], ps[:, 0:512])
                nc.vector.tensor_copy(stage2[:, ic, 512:1024], ps[:, 512:1024])
            else:
                nc.vector.tensor_copy(stage2[:, ic, 0:512], ps[:, 0:512])
                nc.scalar.copy(stage2[:, ic, 512:1024], ps[:, 512:1024])
            if ic % 2 == 1:
                nc.sync.dma_start(
                    outr_r[:, ic - 1 : ic + 1, :], stage2[:, ic - 1 : ic + 1, :]
                )
        while fillers:
            drain(fillers.popleft()[1])


def _build(reps=1):
    nc = bacc.Bacc("TRN2", target_bir_lowering=False, debug=False)
    xT_d = nc.dram_tensor("xT", [DIM, N], BF16, kind="ExternalInput")
    wq_d = nc.dram_tensor("wq", [NPAIR, P, KC, P], BF16, kind="ExternalInput")
    wk_d = nc.dram_tensor("wk", [NPAIR, P, KC, P], BF16, kind="ExternalInput")
    wv_d = nc.dram_tensor("wv", [NPAIR, P, KC, P], BF16, kind="ExternalInput")
    wo_d = nc.dram_tensor("wo", [P, FC, DIM], BF16, kind="ExternalInput")
    id_d = nc.dram_tensor("ident", [P, P], BF16, kind="ExternalInput")
    out_d = nc.dram_tensor("out", [N, DIM], F16, kind="ExternalOutput")
    outr_d = nc.dram_tensor("outr", [N // 2, DIM], F16, kind="ExternalOutput")

    with nc.allow_low_precision(reason="bf16 operands are intended"):
        with tile.TileContext(nc) as tc:
            for _ in range(reps):
                _emit(nc, tc, xT_d, wq_d, wk_d, wv_d, wo_d, id_d, out_d, outr_d)
    nc.compile()
    return nc


def _get_nc():
    if "nc" not in _CACHE:
        _CACHE["nc"] = _build()
    return _CACHE["nc"]


def _pack_pairs(w):
    # [1024, 512] -> [pair, p, kc, c] with (kc, c) contiguous per partition
    return np.ascontiguousarray(
        w.reshape(KC, P, NPAIR, P).transpose(2, 1, 0, 3)
    ).astype(bfloat16)


def kernel(x, w_qkv, w_out, b_out, _trace=False, _tmpdir=None):
    x = np.asarray(x, dtype=np.float32)
    w_qkv = np.asarray(w_qkv, dtype=np.float32)
    w_out = np.asarray(w_out, dtype=np.float32)
    b_out = np.asarray(b_out, dtype=np.float32)

    nc = _get_nc()
    ident = np.eye(P, dtype=bfloat16)
    in_maps = []
    for j in range(8):
        b, hg = j // 2, j % 2
        s = FEAT * hg
        wo_h = np.ascontiguousarray(
            w_out[s : s + FEAT, :].reshape(FC, P, DIM).transpose(1, 0, 2)
        ).astype(bfloat16)
        in_maps.append(
            {
                "xT": np.ascontiguousarray(x[b].T).astype(bfloat16),
                "wq": _pack_pairs(w_qkv[:, s : s + FEAT]),
                "wk": _pack_pairs(w_qkv[:, DIM + s : DIM + s + FEAT]),
                "wv": _pack_pairs(w_qkv[:, 2 * DIM + s : 2 * DIM + s + FEAT]),
                "wo": wo_h,
                "ident": ident,
            }
        )
    res = run_bass_kernel_spmd(
        nc, in_maps, core_ids=list(range(8)), trace=_trace, tmpdir=_tmpdir
    )
    out = np.empty((B, N, DIM), np.float32)
    for b in range(B):
        acc = None
        for j in (2 * b, 2 * b + 1):
            o = res.results[j]["out"].astype(np.float32)
            o[N // 2 :] += res.results[j]["outr"].astype(np.float32)
            acc = o if acc is None else acc + o
        out[b] = acc
    out += b_out[None, None, :]
    if _trace:
        return out, res
    return out


# revision 12
# speedup vs baseline: 1.0686x; 1.0686x over previous
"""TRN2 Bass kernel for nn_Attention_56281251447235 (v3.2).

Multi-head attention: x:[4,2048,1024], w_qkv:[1024,3072] (q|k|v),
16 heads x 64 dim_head, w_out:[1024,1024], b_out:[1024].

Sharding over 8 NeuronCores: core j handles batch b=j//2 and head-group
hg=j%2 (8 of 16 heads).  Each core computes its 8 heads' attention and the
head-group's output projection [2048,1024] in f16; the host sums the two
head-group arrays per batch (plus a small tail-residual array) and adds
the bias.

Design (cost-model driven):
  * The ACT engine (exp) is the pacer: 256 x 1038ns activations.  The
    Tile scheduler is a per-engine greedy list scheduler keyed on
    bass_priority (emission order), so all "filler" work - q/k/v
    projections, output projection, transposes, stores - is emitted in a
    BACKGROUND priority band: it fills PE idle slots without ever
    delaying the exp-feeding QK matmuls.
  * lookahead ST + lagged PV: per position g the emission order is
    exp(g), ST(g+1), PV(g-1), so when exp(g-1)'s read-release fires,
    the scheduler starts ST(g+1) first (lower priority than PV(g-1))
    and the next exp is never input-starved.
  * dummy activation at t=0 preloads the exp table set off the critical
    path (-1.3us).
  * batched, host-packed input DMAs (pair-packed wq/wk/wv with 2KB
    descriptor runs; x in quarter chunks, head-critical first).
  * v-projection merged per (pair, tci): 8 matmuls of 128-wide.
  * single f16 output: the fc3 (pair-3) projection adds in-place into
    the staged fc0-2 part on-device for tokens 0-1023; for tokens
    1024-2047 the fc3 part ships as a separate f16 residual computed in
    a short per-ic tail pipeline (psum->sbuf copies split over ACT,
    which is idle after the last exp, and DVE).  Stores are batched.
"""

import numpy as np
from ml_dtypes import bfloat16

import concourse.mybir as mybir
import concourse.tile as tile
from concourse import bacc
from concourse.bass_utils import run_bass_kernel_spmd

F32 = mybir.dt.float32
F16 = mybir.dt.float16
BF16 = mybir.dt.bfloat16
EXP = mybir.ActivationFunctionType.Exp

P = 128
B, N, DIM = 4, 2048, 1024
H_LOC = 8  # heads per core
D = 64  # dim per head
FEAT = H_LOC * D  # 512 inner dims per core
KC = DIM // P  # 8 contraction chunks over model dim
NT = N // P  # 16 token chunks
FC = FEAT // P  # 4 feature chunks
IB = 1024  # attention i-block width
NIB = N // IB  # 2
NPAIR = H_LOC // 2  # 4
SCALE = 1.0 / 8.0  # dim_head ** -0.5
BG = 1 << 22  # background priority band offset

_CACHE = {}

import os as _os


def _emit(nc, tc, xT_d, wq_d, wk_d, wv_d, wo_d, id_d, out_d, outr_d):
    from contextlib import ExitStack, contextmanager
    from collections import deque

    @contextmanager
    def background():
        save = tc.cur_priority
        tc.cur_priority = save + BG
        try:
            yield
        finally:
            tc.cur_priority = save

    with ExitStack() as ctx:
        big = ctx.enter_context(tc.tile_pool(name="big", bufs=1))
        mm512 = ctx.enter_context(tc.tile_pool(name="mm512", bufs=2, space="PSUM"))
        ps_st = ctx.enter_context(tc.tile_pool(name="ps_st", bufs=2, space="PSUM"))
        ps_ot = ctx.enter_context(tc.tile_pool(name="ps_ot", bufs=1, space="PSUM"))
        pb2 = ctx.enter_context(tc.tile_pool(name="pb2", bufs=4))  # qT/kT, all pairs
        pb1 = ctx.enter_context(tc.tile_pool(name="pb1", bufs=1))  # weights
        pbe = ctx.enter_context(tc.tile_pool(name="pbe", bufs=4))  # ex ring

        # ---- persistent tiles ----
        xT = big.tile([P, KC, N], BF16)  # 32KB/p
        v_aug = big.tile([P, NT, H_LOC, D + 1], BF16)  # 16.6KB/p
        OT = big.tile([P, FC, N], BF16)  # 16KB/p
        ident = big.tile([P, P], BF16)
        rec_sb = big.tile([P, 2, 4, 1], F32)
        o_all = big.tile([P, NIB, FC, 8, 2, D], BF16)  # 16KB/p
        stage = big.tile([P, NT, DIM], F16)  # 32KB/p - final output staging
        stage2 = big.tile([P, NT // 2, DIM], F16)  # 16KB/p - fc3 tail residual
        dummy = big.tile([P, 2], F32)
        wv = pb1.tile([P, KC, FEAT], BF16, tag="wv")
        wo = pb1.tile([P, FC, DIM], BF16, tag="wo")

        # dummy activation: pulls the exp table load to t=0 (off the
        # critical path; the table DMA is ~1.3us)
        nc.vector.memset(dummy[:], 0.0)
        nc.scalar.activation(dummy[:, 1:2], dummy[:, 0:1], EXP, scale=1.0)

        # PE warmup: garbage matmuls in the deepest background band keep
        # the PE p-state ramp alive through the DMA-paced head (the cost
        # model runs matmuls at half clock until ~3us of continuous use)
        warm = big.tile([P, 512], BF16)
        nc.vector.memset(warm[:], 0.0)
        save_prio = tc.cur_priority
        tc.cur_priority = save_prio + 2 * BG
        for _ in range(10):
            wps = mm512.tile([P, 512], F32, tag="mm512", name="wps")
            nc.tensor.matmul(
                wps[0:64, :], warm[0:64, 0:64], warm[0:64, :], start=True, stop=True
            )
        tc.cur_priority = save_prio

        # ones column of v_aug (65th col of every head) via f32 scratch
        with tc.tile_pool(name="init", bufs=1) as init:
            onec = init.tile([P, 1, 1], F32)
            nc.vector.memset(onec[:], 1.0)
            nc.vector.tensor_copy(
                v_aug[:, :, :, D], onec[:].to_broadcast([P, NT, H_LOC])
            )

        # ---- input DMAs (order = head-critical first) ----
        wqs, wks = [], []
        for pair in range(NPAIR):
            wqs.append(pb1.tile([P, KC, P], BF16, tag=f"wq{pair}", name=f"wq{pair}"))
            wks.append(pb1.tile([P, KC, P], BF16, tag=f"wk{pair}", name=f"wk{pair}"))
        xT_r = xT_d.ap().rearrange("(kc p) t -> p kc t", p=P)
        out_r = out_d.ap().rearrange("(tc p) o -> p tc o", p=P)
        outr_r = outr_d.ap().rearrange("(tc p) o -> p tc o", p=P)

        # head: k/q weights + x token-half-0 first; the first k-proj
        # matmul starts ~4.5us and exp0 ~11.5us
        nc.sync.dma_start(wks[0][:, 0:2], wk_d.ap()[0, :, 0:2])
        nc.sync.dma_start(xT[:, 0:4, 0:512], xT_r[:, 0:4, 0:512])
        nc.sync.dma_start(wks[0][:, 2:8], wk_d.ap()[0, :, 2:8])
        nc.sync.dma_start(wqs[0][:], wq_d.ap()[0])
        nc.sync.dma_start(xT[:, 4:8, 0:512], xT_r[:, 4:8, 0:512])
        nc.sync.dma_start(xT[:, 0:4, 512:1024], xT_r[:, 0:4, 512:1024])
        nc.sync.dma_start(xT[:, 4:8, 512:1024], xT_r[:, 4:8, 512:1024])
        nc.sync.dma_start(wv[:, :, 0:P], wv_d.ap()[0])  # pair-0 v weights
        nc.sync.dma_start(ident[:], id_d.ap())
        nc.sync.dma_start(xT[:, :, 1024:1536], xT_r[:, :, 1024:1536])
        nc.sync.dma_start(wv[:, :, P : 2 * P], wv_d.ap()[1])
        nc.sync.dma_start(wks[1][:], wk_d.ap()[1])
        nc.sync.dma_start(wqs[1][:], wq_d.ap()[1])
        nc.sync.dma_start(xT[:, :, 1536:2048], xT_r[:, :, 1536:2048])
        nc.sync.dma_start(wv[:, :, 2 * P : 3 * P], wv_d.ap()[2])
        nc.sync.dma_start(wks[2][:], wk_d.ap()[2])
        nc.sync.dma_start(wqs[2][:], wq_d.ap()[2])
        nc.sync.dma_start(wv[:, :, 3 * P : 4 * P], wv_d.ap()[3])
        nc.sync.dma_start(wks[3][:], wk_d.ap()[3])
        nc.sync.dma_start(wqs[3][:], wq_d.ap()[3])
        nc.sync.dma_start(wo[:], wo_d.ap())

        # ---- work units (direct emitters) ----
        def proj_unit(dst, w, ib4):
            ps = mm512.tile([P, 512], F32, tag="mm512", name="ps")
            for kc in range(KC):
                nc.tensor.matmul(
                    ps[:],
                    w[:, kc],
                    xT[:, kc, ib4 * 512 : (ib4 + 1) * 512],
                    start=(kc == 0),
                    stop=(kc == KC - 1),
                )
            nc.vector.tensor_copy(dst[:, ib4 * 512 : (ib4 + 1) * 512], ps[:])

        def v_unit(pair, tci):
            # both heads of `pair` at once: moving 128 wide
            ps = mm512.tile([P, 2 * D], F32, tag="mm512", name="ps")
            for kc in range(KC):
                nc.tensor.matmul(
                    ps[:],
                    xT[:, kc, tci * P : (tci + 1) * P],
                    wv[:, kc, pair * P : (pair + 1) * P],
                    start=(kc == 0),
                    stop=(kc == KC - 1),
                )
            nc.vector.tensor_copy(
                v_aug[:, tci, 2 * pair : 2 * pair + 2, 0:D],
                ps[:].rearrange("p (h d) -> p h d", h=2),
            )

        def ca_unit(tci, nb):
            # fc0-2 output projection -> staging (f16)
            ps = mm512.tile([P, 512], F32, tag="mm512", name="ps")
            for i, fc in enumerate((0, 1, 2)):
                nc.tensor.matmul(
                    ps[:],
                    OT[:, fc, tci * P : (tci + 1) * P],
                    wo[:, fc, nb * 512 : (nb + 1) * 512],
                    start=(i == 0),
                    stop=(i == 2),
                )
            nc.vector.tensor_copy(stage[:, tci, nb * 512 : (nb + 1) * 512], ps[:])

        def cb_unit(tci, nb):
            # fc3 projection, added in place into the staged fc0-2 part
            ps = mm512.tile([P, 512], F32, tag="mm512", name="ps")
            nc.tensor.matmul(
                ps[:],
                OT[:, 3, tci * P : (tci + 1) * P],
                wo[:, 3, nb * 512 : (nb + 1) * 512],
                start=True,
                stop=True,
            )
            sl = stage[:, tci, nb * 512 : (nb + 1) * 512]
            nc.vector.tensor_add(sl, ps[:], sl)

        def tp_unit(pair, ib):
            op = o_all[:, ib, pair]
            tp = mm512.tile([P, 8, P], BF16, tag="mm512", name="tp")
            for ic in range(8):
                nc.tensor.matmul(
                    tp[:, ic],
                    op[:, ic].rearrange("p a b -> p (a b)"),
                    ident[:],
                    is_transpose=True,
                    start=(ic == 0),
                    stop=(ic == 7),
                )
            nc.vector.tensor_copy(
                OT[:, pair, ib * IB : (ib + 1) * IB],
                tp[:].rearrange("p a b -> p (a b)"),
            )

        qTs, kTs = [], []
        for pair in range(NPAIR):
            qTs.append(pb2.tile([P, N], BF16, tag="qT", name="qT"))
            kTs.append(pb2.tile([P, N], BF16, tag="kT", name="kT"))

        # ---- phase A: eager prefix on the main band ----
        proj_unit(kTs[0], wks[0], 0)
        proj_unit(qTs[0], wqs[0], 0)
        proj_unit(qTs[0], wqs[0], 1)
        for tci in range(8):
            v_unit(0, tci)
        proj_unit(kTs[0], wks[0], 1)

        # ---- all projection/v work in the background band, in
        # consumption order; the scheduler runs it in PE idle slots ----
        with background():
            for pair in range(NPAIR):
                if pair > 0:
                    proj_unit(kTs[pair], wks[pair], 0)
                    proj_unit(qTs[pair], wqs[pair], 0)
                    proj_unit(qTs[pair], wqs[pair], 1)
                    for tci in range(8):
                        v_unit(pair, tci)
                    proj_unit(kTs[pair], wks[pair], 1)
                proj_unit(kTs[pair], wks[pair], 2)
                for tci in range(8, 12):
                    v_unit(pair, tci)
                proj_unit(kTs[pair], wks[pair], 3)
                for tci in range(12, 16):
                    v_unit(pair, tci)
                proj_unit(qTs[pair], wqs[pair], 2)
                proj_unit(qTs[pair], wqs[pair], 3)

        # ---- main sweep: lookahead-ST, lagged-PV ACT-paced stream ----
        blocks = [(p, i, h) for p in range(NPAIR) for i in range(NIB) for h in range(2)]
        positions = [(bi, jc) for bi in range(len(blocks)) for jc in range(NT)]

        st_q = deque()

        def emit_st_for(bi, jc):
            pair, ib, h2 = blocks[bi]
            st = ps_st.tile([P, IB], F32, tag="st", name="st")
            kh = kTs[pair][h2 * D : (h2 + 1) * D]
            qh = qTs[pair][h2 * D : (h2 + 1) * D]
            for hf in range(2):
                nc.tensor.matmul(
                    st[:, hf * 512 : (hf + 1) * 512],
                    kh[:, jc * P : (jc + 1) * P],
                    qh[:, ib * IB + hf * 512 : ib * IB + (hf + 1) * 512],
                    start=True,
                    stop=True,
                )
            st_q.append(st)

        emit_st_for(0, 0)

        ot_state = {"tile": None}

        def emit_pv(bi, jc, ex):
            pair, ib, h2 = blocks[bi]
            h = 2 * pair + h2
            if jc == 0:
                ot_state["tile"] = ps_ot.tile([P, 2, 4, P], F32, tag="ot", name="ot_ps")
            ot_ps = ot_state["tile"]
            for ic in range(8):
                nc.tensor.matmul(
                    ot_ps[:, ic // 4, ic % 4, 0 : D + 1],
                    ex[:, ic * P : (ic + 1) * P],
                    v_aug[:, jc, h],
                    start=(jc == 0 and ic % 4 == 0),
                    stop=(jc == NT - 1 and ic % 4 == 3),
                )
            if jc == NT - 1:
                # normalization of this block; DVE, overlaps the next exps
                nc.vector.reciprocal(rec_sb[:], ot_ps[:, :, :, D : D + 1])
                for b2 in range(2):
                    nc.vector.tensor_mul(
                        o_all[:, ib, pair, b2 * 4 : (b2 + 1) * 4, h2, :],
                        ot_ps[:, b2, :, 0:D],
                        rec_sb[:, b2].to_broadcast([P, 4, D]),
                    )
                if h2 == 1 and not (pair == 3 and ib == 1):
                    with background():
                        tp_unit(pair, ib)

        lag = None  # (bi, jc, ex) whose PV is not yet emitted
        for idx, (bi, jc) in enumerate(positions):
            # ACT: exp of the pre-emitted ST
            ex = pbe.tile([P, IB], BF16, tag="ex", name="ex", bufs=6)
            nc.scalar.activation(ex[:], st_q.popleft()[:], EXP, scale=SCALE)

            # PE: lookahead ST for the next position
            if idx + 1 < len(positions):
                emit_st_for(*positions[idx + 1])

            # PE: lagged PV (previous position); emits norm/tp at jc==15
            if lag is not None:
                emit_pv(*lag)
            lag = (bi, jc, ex)

            # output-projection units, queued after the tp they depend on
            if jc == 0:
                with background():
                    if bi == 10:
                        for tci in range(8):
                            for nb in range(2):
                                ca_unit(tci, nb)
                    if bi == 12:
                        for tci in range(8, 12):
                            for nb in range(2):
                                ca_unit(tci, nb)
                        nc.sync.dma_start(out_r[:, 8:12, :], stage[:, 8:12, :])
                        for tci in range(12, 16):
                            for nb in range(2):
                                ca_unit(tci, nb)
                        nc.sync.dma_start(out_r[:, 12:16, :], stage[:, 12:16, :])
                    if bi == 14:
                        for tci in range(4):
                            for nb in range(2):
                                cb_unit(tci, nb)
                        nc.sync.dma_start(out_r[:, 0:4, :], stage[:, 0:4, :])
                        for tci in range(4, 8):
                            for nb in range(2):
                                cb_unit(tci, nb)
                        nc.sync.dma_start(out_r[:, 4:8, :], stage[:, 4:8, :])

        # ---- tail ----
        emit_pv(*lag)  # PV(15,15) + norm; tp3.1 handled per-ic below
        op = o_all[:, 1, 3]
        for ic in range(8):
            tp = mm512.tile([P, P], BF16, tag="mm512", name="tp")
            nc.tensor.matmul(
                tp[:],
                op[:, ic].rearrange("p a b -> p (a b)"),
                ident[:],
                is_transpose=True,
                start=True,
                stop=True,
            )
            otc = OT[:, 3, IB + ic * P : IB + (ic + 1) * P]
            if ic % 2 == 0:
                nc.scalar.copy(otc, tp[:])
            else:
                nc.vector.tensor_copy(otc, tp[:])
            if ic % 3 == 2:
                pst = ps_ot.tile([P, 2, 4, P], F32, tag="ot", name="ot_ps")
                ps = pst[:].rearrange("p a b c -> p (a b c)")
            else:
                pst = ps_st.tile([P, IB], F32, tag="st", name="st")
                ps = pst[:]
            for nb in range(2):
                nc.tensor.matmul(
                    ps[:, nb * 512 : (nb + 1) * 512],
                    otc,
                    wo[:, 3, nb * 512 : (nb + 1) * 512],
                    start=True,
                    stop=True,
                )
            # psum->sbuf copies split over ACT (idle after the last exp)
            # and DVE so neither engine paces the tail alone
            nc.scalar.copy(stage2[:, ic, 0:512], ps[:, 0:512])
            nc.vector.tensor_copy(stage2[:, ic, 512:1024], ps[:, 512:1024])
            if ic in (1, 3, 5):
                nc.sync.dma_start(
                    outr_r[:, ic - 1 : ic + 1, :], stage2[:, ic - 1 : ic + 1, :]
                )
            elif ic >= 6:
                nc.sync.dma_start(
                    outr_r[:, ic : ic + 1, :], stage2[:, ic : ic + 1, :]
                )


def _build(reps=1):
    nc = bacc.Bacc("TRN2", target_bir_lowering=False, debug=False)
    xT_d = nc.dram_tensor("xT", [DIM, N], BF16, kind="ExternalInput")
    wq_d = nc.dram_tensor("wq", [NPAIR, P, KC, P], BF16, kind="ExternalInput")
    wk_d = nc.dram_tensor("wk", [NPAIR, P, KC, P], BF16, kind="ExternalInput")
    wv_d = nc.dram_tensor("wv", [NPAIR, P, KC, P], BF16, kind="ExternalInput")
    wo_d = nc.dram_tensor("wo", [P, FC, DIM], BF16, kind="ExternalInput")
    id_d = nc.dram_tensor("ident", [P, P], BF16, kind="ExternalInput")
    out_d = nc.dram_tensor("out", [N, DIM], F16, kind="ExternalOutput")
    outr_d = nc.dram_tensor("outr", [N // 2, DIM], F16, kind="ExternalOutput")

    with nc.allow_low_precision(reason="bf16 operands are intended"):
        with tile.TileContext(nc) as tc:
            for _ in range(reps):
                _emit(nc, tc, xT_d, wq_d, wk_d, wv_d, wo_d, id_d, out_d, outr_d)
    nc.compile()
    return nc


def _get_nc():
    if "nc" not in _CACHE:
        _CACHE["nc"] = _build()
    return _CACHE["nc"]


def _pack_pairs(w):
    # [1024, 512] -> [pair, p, kc, c] with (kc, c) contiguous per partition
    return np.ascontiguousarray(
        w.reshape(KC, P, NPAIR, P).transpose(2, 1, 0, 3)
    ).astype(bfloat16)


def kernel(x, w_qkv, w_out, b_out, _trace=False, _tmpdir=None):
    x = np.asarray(x, dtype=np.float32)
    w_qkv = np.asarray(w_qkv, dtype=np.float32)
    w_out = np.asarray(w_out, dtype=np.float32)
    b_out = np.asarray(b_out, dtype=np.float32)

    nc = _get_nc()
    ident = np.eye(P, dtype=bfloat16)
    in_maps = []
    for j in range(8):
        b, hg = j // 2, j % 2
        s = FEAT * hg
        wo_h = np.ascontiguousarray(
            w_out[s : s + FEAT, :].reshape(FC, P, DIM).transpose(1, 0, 2)
        ).astype(bfloat16)
        in_maps.append(
            {
                "xT": np.ascontiguousarray(x[b].T).astype(bfloat16),
                "wq": _pack_pairs(w_qkv[:, s : s + FEAT]),
                "wk": _pack_pairs(w_qkv[:, DIM + s : DIM + s + FEAT]),
                "wv": _pack_pairs(w_qkv[:, 2 * DIM + s : 2 * DIM + s + FEAT]),
                "wo": wo_h,
                "ident": ident,
            }
        )
    res = run_bass_kernel_spmd(
        nc, in_maps, core_ids=list(range(8)), trace=_trace, tmpdir=_tmpdir
    )
    out = np.empty((B, N, DIM), np.float32)
    for b in range(B):
        acc = None
        for j in (2 * b, 2 * b + 1):
            o = res.results[j]["out"].astype(np.float32)
            o[N // 2 :] += res.results[j]["outr"].astype(np.float32)
            acc = o if acc is None else acc + o
        out[b] = acc
    out += b_out[None, None, :]
    if _trace:
        return out, res
    return out


# revision 18
# speedup vs baseline: 1.0725x; 1.0036x over previous
"""TRN2 Bass kernel for nn_Attention_56281251447235 (v3.2).

Multi-head attention: x:[4,2048,1024], w_qkv:[1024,3072] (q|k|v),
16 heads x 64 dim_head, w_out:[1024,1024], b_out:[1024].

Sharding over 8 NeuronCores: core j handles batch b=j//2 and head-group
hg=j%2 (8 of 16 heads).  Each core computes its 8 heads' attention and the
head-group's output projection [2048,1024] in f16; the host sums the two
head-group arrays per batch (plus a small tail-residual array) and adds
the bias.

Design (cost-model driven):
  * The ACT engine (exp) is the pacer: 256 x 1038ns activations.  The
    Tile scheduler is a per-engine greedy list scheduler keyed on
    bass_priority (emission order), so all "filler" work - q/k/v
    projections, output projection, transposes, stores - is emitted in a
    BACKGROUND priority band: it fills PE idle slots without ever
    delaying the exp-feeding QK matmuls.
  * lookahead ST + lagged PV: per position g the emission order is
    exp(g), ST(g+1), PV(g-1), so when exp(g-1)'s read-release fires,
    the scheduler starts ST(g+1) first (lower priority than PV(g-1))
    and the next exp is never input-starved.
  * dummy activation at t=0 preloads the exp table set off the critical
    path (-1.3us).
  * batched, host-packed input DMAs (pair-packed wq/wk/wv with 2KB
    descriptor runs; x in quarter chunks, head-critical first).
  * v-projection merged per (pair, tci): 8 matmuls of 128-wide.
  * single f16 output: the fc3 (pair-3) projection adds in-place into
    the staged fc0-2 part on-device for tokens 0-1023; for tokens
    1024-2047 the fc3 part ships as a separate f16 residual computed in
    a short per-ic tail pipeline (psum->sbuf copies split over ACT,
    which is idle after the last exp, and DVE).  Stores are batched.
"""

import numpy as np
from ml_dtypes import bfloat16

import concourse.mybir as mybir
import concourse.tile as tile
from concourse import bacc
from concourse.bass_utils import run_bass_kernel_spmd

F32 = mybir.dt.float32
F16 = mybir.dt.float16
BF16 = mybir.dt.bfloat16
EXP = mybir.ActivationFunctionType.Exp

P = 128
B, N, DIM = 4, 2048, 1024
H_LOC = 8  # heads per core
D = 64  # dim per head
FEAT = H_LOC * D  # 512 inner dims per core
KC = DIM // P  # 8 contraction chunks over model dim
NT = N // P  # 16 token chunks
FC = FEAT // P  # 4 feature chunks
IB = 1024  # attention i-block width
NIB = N // IB  # 2
NPAIR = H_LOC // 2  # 4
SCALE = 1.0 / 8.0  # dim_head ** -0.5
BG = 1 << 22  # background priority band offset

_CACHE = {}

import os as _os


def _emit(nc, tc, xT_d, wq_d, wk_d, wv_d, wo_d, id_d, out_d, outr_d):
    from contextlib import ExitStack, contextmanager
    from collections import deque

    @contextmanager
    def background():
        save = tc.cur_priority
        tc.cur_priority = save + BG
        try:
            yield
        finally:
            tc.cur_priority = save

    with ExitStack() as ctx:
        big = ctx.enter_context(tc.tile_pool(name="big", bufs=1))
        mm512 = ctx.enter_context(tc.tile_pool(name="mm512", bufs=2, space="PSUM"))
        ps_st = ctx.enter_context(tc.tile_pool(name="ps_st", bufs=2, space="PSUM"))
        ps_ot = ctx.enter_context(tc.tile_pool(name="ps_ot", bufs=1, space="PSUM"))
        pb2 = ctx.enter_context(tc.tile_pool(name="pb2", bufs=4))  # qT/kT, all pairs
        pb1 = ctx.enter_context(tc.tile_pool(name="pb1", bufs=1))  # weights
        pbe = ctx.enter_context(tc.tile_pool(name="pbe", bufs=4))  # ex ring

        # ---- persistent tiles ----
        xT = big.tile([P, KC, N], BF16)  # 32KB/p
        v_aug = big.tile([P, NT, H_LOC, D + 1], BF16)  # 16.6KB/p
        OT = big.tile([P, FC, N], BF16)  # 16KB/p
        ident = big.tile([P, P], BF16)
        rec_sb = big.tile([P, 2, 4, 1], F32)
        o_all = big.tile([P, NIB, FC, 8, 2, D], BF16)  # 16KB/p
        stage = big.tile([P, NT, DIM], F16)  # 32KB/p - final output staging
        stage2 = big.tile([P, NT // 2, DIM], F16)  # 16KB/p - fc3 tail residual
        dummy = big.tile([P, 2], F32)
        wv = pb1.tile([P, KC, FEAT], BF16, tag="wv")
        wo = pb1.tile([P, FC, DIM], BF16, tag="wo")

        # dummy activation: pulls the exp table load to t=0 (off the
        # critical path; the table DMA is ~1.3us)
        nc.vector.memset(dummy[:], 0.0)
        nc.scalar.activation(dummy[:, 1:2], dummy[:, 0:1], EXP, scale=1.0)

        # PE warmup: garbage matmuls in the deepest background band keep
        # the PE p-state ramp alive through the DMA-paced head (the cost
        # model runs matmuls at half clock until ~3us of continuous use)
        warm = big.tile([P, 512], BF16)
        nc.vector.memset(warm[:], 0.0)
        save_prio = tc.cur_priority
        tc.cur_priority = save_prio + 2 * BG
        for _ in range(10):
            wps = mm512.tile([P, 512], F32, tag="mm512", name="wps")
            nc.tensor.matmul(
                wps[0:64, :], warm[0:64, 0:64], warm[0:64, :], start=True, stop=True
            )
        tc.cur_priority = save_prio

        # ones column of v_aug (65th col of every head) via f32 scratch
        with tc.tile_pool(name="init", bufs=1) as init:
            onec = init.tile([P, 1, 1], F32)
            nc.vector.memset(onec[:], 1.0)
            nc.vector.tensor_copy(
                v_aug[:, :, :, D], onec[:].to_broadcast([P, NT, H_LOC])
            )

        # ---- input DMAs (order = head-critical first) ----
        wqs, wks = [], []
        for pair in range(NPAIR):
            wqs.append(pb1.tile([P, KC, P], BF16, tag=f"wq{pair}", name=f"wq{pair}"))
            wks.append(pb1.tile([P, KC, P], BF16, tag=f"wk{pair}", name=f"wk{pair}"))
        xT_r = xT_d.ap().rearrange("(kc p) t -> p kc t", p=P)
        out_r = out_d.ap().rearrange("(tc p) o -> p tc o", p=P)
        outr_r = outr_d.ap().rearrange("(tc p) o -> p tc o", p=P)

        # head: k/q weights + x token-half-0 first; the first k-proj
        # matmul starts ~4.5us and exp0 ~11.5us
        nc.sync.dma_start(wks[0][:, 0:2], wk_d.ap()[0, :, 0:2])
        nc.sync.dma_start(xT[:, 0:4, 0:512], xT_r[:, 0:4, 0:512])
        nc.sync.dma_start(wks[0][:, 2:8], wk_d.ap()[0, :, 2:8])
        nc.sync.dma_start(wqs[0][:], wq_d.ap()[0])
        nc.sync.dma_start(xT[:, 4:8, 0:512], xT_r[:, 4:8, 0:512])
        nc.sync.dma_start(xT[:, 0:4, 512:1024], xT_r[:, 0:4, 512:1024])
        nc.sync.dma_start(xT[:, 4:8, 512:1024], xT_r[:, 4:8, 512:1024])
        nc.sync.dma_start(wv[:, :, 0:P], wv_d.ap()[0])  # pair-0 v weights
        nc.sync.dma_start(ident[:], id_d.ap())
        nc.sync.dma_start(xT[:, :, 1024:1536], xT_r[:, :, 1024:1536])
        nc.sync.dma_start(wv[:, :, P : 2 * P], wv_d.ap()[1])
        nc.sync.dma_start(wks[1][:], wk_d.ap()[1])
        nc.sync.dma_start(wqs[1][:], wq_d.ap()[1])
        nc.sync.dma_start(xT[:, :, 1536:2048], xT_r[:, :, 1536:2048])
        nc.sync.dma_start(wv[:, :, 2 * P : 3 * P], wv_d.ap()[2])
        nc.sync.dma_start(wks[2][:], wk_d.ap()[2])
        nc.sync.dma_start(wqs[2][:], wq_d.ap()[2])
        nc.sync.dma_start(wv[:, :, 3 * P : 4 * P], wv_d.ap()[3])
        nc.sync.dma_start(wks[3][:], wk_d.ap()[3])
        nc.sync.dma_start(wqs[3][:], wq_d.ap()[3])
        nc.sync.dma_start(wo[:], wo_d.ap())

        # ---- work units (direct emitters) ----
        def proj_unit(dst, w, ib4):
            ps = mm512.tile([P, 512], F32, tag="mm512", name="ps")
            for kc in range(KC):
                nc.tensor.matmul(
                    ps[:],
                    w[:, kc],
                    xT[:, kc, ib4 * 512 : (ib4 + 1) * 512],
                    start=(kc == 0),
                    stop=(kc == KC - 1),
                )
            nc.vector.tensor_copy(dst[:, ib4 * 512 : (ib4 + 1) * 512], ps[:])

        def v_unit(pair, tci):
            # both heads of `pair` at once: moving 128 wide
            ps = mm512.tile([P, 2 * D], F32, tag="mm512", name="ps")
            for kc in range(KC):
                nc.tensor.matmul(
                    ps[:],
                    xT[:, kc, tci * P : (tci + 1) * P],
                    wv[:, kc, pair * P : (pair + 1) * P],
                    start=(kc == 0),
                    stop=(kc == KC - 1),
                )
            nc.vector.tensor_copy(
                v_aug[:, tci, 2 * pair : 2 * pair + 2, 0:D],
                ps[:].rearrange("p (h d) -> p h d", h=2),
            )

        def ca_unit(tci, nb):
            # fc0-2 output projection -> staging (f16)
            ps = mm512.tile([P, 512], F32, tag="mm512", name="ps")
            for i, fc in enumerate((0, 1, 2)):
                nc.tensor.matmul(
                    ps[:],
                    OT[:, fc, tci * P : (tci + 1) * P],
                    wo[:, fc, nb * 512 : (nb + 1) * 512],
                    start=(i == 0),
                    stop=(i == 2),
                )
            nc.vector.tensor_copy(stage[:, tci, nb * 512 : (nb + 1) * 512], ps[:])

        def cb_unit(tci, nb):
            # fc3 projection, added in place into the staged fc0-2 part
            ps = mm512.tile([P, 512], F32, tag="mm512", name="ps")
            nc.tensor.matmul(
                ps[:],
                OT[:, 3, tci * P : (tci + 1) * P],
                wo[:, 3, nb * 512 : (nb + 1) * 512],
                start=True,
                stop=True,
            )
            sl = stage[:, tci, nb * 512 : (nb + 1) * 512]
            nc.vector.tensor_add(sl, ps[:], sl)

        def tp_unit(pair, ib):
            # xbar DMA transpose: SBUF->SBUF, per 128x128 ic-chunk; frees
            # ~1024 PE cycles/unit (PE is the binding engine mid-kernel)
            op = o_all[:, ib, pair]
            for ic in range(8):
                nc.sync.dma_start_transpose(
                    OT[:, pair, ib * IB + ic * P : ib * IB + (ic + 1) * P],
                    op[:, ic].rearrange("p a b -> p (a b)"),
                )

        qTs, kTs = [], []
        for pair in range(NPAIR):
            qTs.append(pb2.tile([P, N], BF16, tag="qT", name="qT"))
            kTs.append(pb2.tile([P, N], BF16, tag="kT", name="kT"))

        # ---- phase A: eager prefix on the main band ----
        proj_unit(kTs[0], wks[0], 0)
        proj_unit(qTs[0], wqs[0], 0)
        proj_unit(qTs[0], wqs[0], 1)
        for tci in range(8):
            v_unit(0, tci)
        proj_unit(kTs[0], wks[0], 1)

        # ---- all projection/v work in the background band, in
        # consumption order; the scheduler runs it in PE idle slots ----
        with background():
            for pair in range(NPAIR):
                if pair > 0:
                    proj_unit(kTs[pair], wks[pair], 0)
                    proj_unit(qTs[pair], wqs[pair], 0)
                    proj_unit(qTs[pair], wqs[pair], 1)
                    for tci in range(8):
                        v_unit(pair, tci)
                    proj_unit(kTs[pair], wks[pair], 1)
                proj_unit(kTs[pair], wks[pair], 2)
                for tci in range(8, 12):
                    v_unit(pair, tci)
                proj_unit(kTs[pair], wks[pair], 3)
                for tci in range(12, 16):
                    v_unit(pair, tci)
                proj_unit(qTs[pair], wqs[pair], 2)
                proj_unit(qTs[pair], wqs[pair], 3)

        # ---- main sweep: lookahead-ST, lagged-PV ACT-paced stream ----
        # Items are (bi, jc, hf): hf=None is a full 1024-wide position;
        # hf=0/1 are 512-wide halves (used for block0 jc0..5 so the first
        # exps only need q0.0, which lands ~4us before q0.1).
        blocks = [(p, i, h) for p in range(NPAIR) for i in range(NIB) for h in range(2)]
        NSPL = int(_os.environ.get('NSPL', '0'))
        positions = (
            [(0, jc, 0) for jc in range(NSPL)]
            + [(0, jc, 1) for jc in range(NSPL)]
            + [(0, jc, None) for jc in range(NSPL, NT)]
            + [(bi, jc, None) for bi in range(1, len(blocks)) for jc in range(NT)]
        )

        st_q = deque()

        def emit_st_for(bi, jc, hf):
            pair, ib, h2 = blocks[bi]
            kh = kTs[pair][h2 * D : (h2 + 1) * D]
            qh = qTs[pair][h2 * D : (h2 + 1) * D]
            if hf is None:
                st = ps_st.tile([P, IB], F32, tag="st", name="st")
                for h in range(2):
                    nc.tensor.matmul(
                        st[:, h * 512 : (h + 1) * 512],
                        kh[:, jc * P : (jc + 1) * P],
                        qh[:, ib * IB + h * 512 : ib * IB + (h + 1) * 512],
                        start=True,
                        stop=True,
                    )
            else:
                st = ps_st.tile([P, 512], F32, tag="st", name="st")
                nc.tensor.matmul(
                    st[:],
                    kh[:, jc * P : (jc + 1) * P],
                    qh[:, ib * IB + hf * 512 : ib * IB + (hf + 1) * 512],
                    start=True,
                    stop=True,
                )
            st_q.append(st)

        emit_st_for(*positions[0])

        ot_state = {"tile": None}

        def emit_pv(bi, jc, hf, ex):
            pair, ib, h2 = blocks[bi]
            h = 2 * pair + h2
            if jc == 0 and hf in (None, 0):
                ot_state["tile"] = ps_ot.tile([P, 2, 4, P], F32, tag="ot", name="ot_ps")
            ot_ps = ot_state["tile"]
            ics = range(8) if hf is None else range(4 * hf, 4 * hf + 4)
            for ic in ics:
                exc = ex[:, (ic - (0 if hf is None else 4 * hf)) * P :][:, 0:P]
                nc.tensor.matmul(
                    ot_ps[:, ic // 4, ic % 4, 0 : D + 1],
                    exc,
                    v_aug[:, jc, h],
                    start=(jc == 0 and ic % 4 == 0),
                    stop=(jc == NT - 1 and ic % 4 == 3),
                )
            if jc == NT - 1 and hf in (None, 1):
                # normalization of this block; DVE, overlaps the next exps
                nc.vector.reciprocal(rec_sb[:], ot_ps[:, :, :, D : D + 1])
                for b2 in range(2):
                    nc.vector.tensor_mul(
                        o_all[:, ib, pair, b2 * 4 : (b2 + 1) * 4, h2, :],
                        ot_ps[:, b2, :, 0:D],
                        rec_sb[:, b2].to_broadcast([P, 4, D]),
                    )
                if h2 == 1 and not (pair == 3 and ib == 1):
                    with background():
                        tp_unit(pair, ib)

        lag = None  # (bi, jc, hf, ex) whose PV is not yet emitted
        for idx, (bi, jc, hf) in enumerate(positions):
            # ACT: exp of the pre-emitted ST
            w = IB if hf is None else 512
            ex = pbe.tile([P, w], BF16, tag="ex", name="ex", bufs=6)
            nc.scalar.activation(ex[:], st_q.popleft()[:], EXP, scale=SCALE)

            # PE: lookahead ST for the next position
            if idx + 1 < len(positions):
                emit_st_for(*positions[idx + 1])

            # PE: lagged PV (previous position); emits norm/tp at jc==15
            if lag is not None:
                emit_pv(*lag)
            lag = (bi, jc, hf, ex)

            # output-projection units, queued after the tp they depend on
            if jc == 0:
                with background():
                    if bi == 10:
                        for tci in range(8):
                            for nb in range(2):
                                ca_unit(tci, nb)
                    if bi == 12:
                        for tci in range(8, 12):
                            for nb in range(2):
                                ca_unit(tci, nb)
                        nc.sync.dma_start(out_r[:, 8:12, :], stage[:, 8:12, :])
                        for tci in range(12, 16):
                            for nb in range(2):
                                ca_unit(tci, nb)
                        nc.sync.dma_start(out_r[:, 12:16, :], stage[:, 12:16, :])
                    if bi == 14:
                        for tci in range(4):
                            for nb in range(2):
                                cb_unit(tci, nb)
                        nc.sync.dma_start(out_r[:, 0:4, :], stage[:, 0:4, :])
                        for tci in range(4, 8):
                            for nb in range(2):
                                cb_unit(tci, nb)
                        nc.sync.dma_start(out_r[:, 4:8, :], stage[:, 4:8, :])

        # ---- tail ----
        emit_pv(*lag)  # PV(15,15) + norm; tp3.1 handled per-ic below
        op = o_all[:, 1, 3]
        for ic in range(8):
            tp = mm512.tile([P, P], BF16, tag="mm512", name="tp")
            nc.tensor.matmul(
                tp[:],
                op[:, ic].rearrange("p a b -> p (a b)"),
                ident[:],
                is_transpose=True,
                start=True,
                stop=True,
            )
            otc = OT[:, 3, IB + ic * P : IB + (ic + 1) * P]
            if ic % 2 == 0:
                nc.scalar.copy(otc, tp[:])
            else:
                nc.vector.tensor_copy(otc, tp[:])
            if ic % 3 == 2:
                pst = ps_ot.tile([P, 2, 4, P], F32, tag="ot", name="ot_ps")
                ps = pst[:].rearrange("p a b c -> p (a b c)")
            else:
                pst = ps_st.tile([P, IB], F32, tag="st", name="st")
                ps = pst[:]
            for nb in range(2):
                nc.tensor.matmul(
                    ps[:, nb * 512 : (nb + 1) * 512],
                    otc,
                    wo[:, 3, nb * 512 : (nb + 1) * 512],
                    start=True,
                    stop=True,
                )
            # psum->sbuf copies split over ACT (idle after the last exp)
            # and DVE so neither engine paces the tail alone
            nc.scalar.copy(stage2[:, ic, 0:512], ps[:, 0:512])
            nc.vector.tensor_copy(stage2[:, ic, 512:1024], ps[:, 512:1024])
            nc.sync.dma_start(outr_r[:, ic : ic + 1, :], stage2[:, ic : ic + 1, :])


def _build(reps=1):
    nc = bacc.Bacc("TRN2", target_bir_lowering=False, debug=False)
    xT_d = nc.dram_tensor("xT", [DIM, N], BF16, kind="ExternalInput")
    wq_d = nc.dram_tensor("wq", [NPAIR, P, KC, P], BF16, kind="ExternalInput")
    wk_d = nc.dram_tensor("wk", [NPAIR, P, KC, P], BF16, kind="ExternalInput")
    wv_d = nc.dram_tensor("wv", [NPAIR, P, KC, P], BF16, kind="ExternalInput")
    wo_d = nc.dram_tensor("wo", [P, FC, DIM], BF16, kind="ExternalInput")
    id_d = nc.dram_tensor("ident", [P, P], BF16, kind="ExternalInput")
    out_d = nc.dram_tensor("out", [N, DIM], F16, kind="ExternalOutput")
    outr_d = nc.dram_tensor("outr", [N // 2, DIM], F16, kind="ExternalOutput")

    with nc.allow_low_precision(reason="bf16 operands are intended"):
        with tile.TileContext(nc) as tc:
            for _ in range(reps):
                _emit(nc, tc, xT_d, wq_d, wk_d, wv_d, wo_d, id_d, out_d, outr_d)
    nc.compile()
    return nc


def _get_nc():
    if "nc" not in _CACHE:
        _CACHE["nc"] = _build()
    return _CACHE["nc"]


def _pack_pairs(w):
    # [1024, 512] -> [pair, p, kc, c] with (kc, c) contiguous per partition
    return np.ascontiguousarray(
        w.reshape(KC, P, NPAIR, P).transpose(2, 1, 0, 3)
    ).astype(bfloat16)


def kernel(x, w_qkv, w_out, b_out, _trace=False, _tmpdir=None):
    x = np.asarray(x, dtype=np.float32)
    w_qkv = np.asarray(w_qkv, dtype=np.float32)
    w_out = np.asarray(w_out, dtype=np.float32)
    b_out = np.asarray(b_out, dtype=np.float32)

    nc = _get_nc()
    ident = np.eye(P, dtype=bfloat16)
    in_maps = []
    for j in range(8):
        b, hg = j // 2, j % 2
        s = FEAT * hg
        wo_h = np.ascontiguousarray(
            w_out[s : s + FEAT, :].reshape(FC, P, DIM).transpose(1, 0, 2)
        ).astype(bfloat16)
        in_maps.append(
            {
                "xT": np.ascontiguousarray(x[b].T).astype(bfloat16),
                "wq": _pack_pairs(w_qkv[:, s : s + FEAT]),
                "wk": _pack_pairs(w_qkv[:, DIM + s : DIM + s + FEAT]),
                "wv": _pack_pairs(w_qkv[:, 2 * DIM + s : 2 * DIM + s + FEAT]),
                "wo": wo_h,
                "ident": ident,
            }
        )
    res = run_bass_kernel_spmd(
        nc, in_maps, core_ids=list(range(8)), trace=_trace, tmpdir=_tmpdir
    )
    out = np.empty((B, N, DIM), np.float32)
    for b in range(B):
        acc = None
        for j in (2 * b, 2 * b + 1):
            o = res.results[j]["out"].astype(np.float32)
            o[N // 2 :] += res.results[j]["outr"].astype(np.float32)
            acc = o if acc is None else acc + o
        out[b] = acc
    out += b_out[None, None, :]
    if _trace:
        return out, res
    return out
